# revision 6
# baseline (speedup 1.0000x reference)
"""AdaGNN on 8 TRN2 NeuronCores (Bass, SPMD).

Strategy (node sharding, replicated graph tables):
- Host packs the 50000 nodes into 8 cores x 49 blocks x 128 rows via a
  load-balancing permutation (block in-degree sums <= TPB*128).
- spmm per 128-slot tile: one K=1 indirect DMA gathers the 128 source rows
  (one per partition) from the DRAM feature table; the DVE builds a
  (128 slots x 128 rows) one-hot-times-val matrix; the PE accumulates
  psum_block += S_vals.T @ gathered.  Graph structure (gather indices,
  dest-locals, vals) is identical for all 4 spmm layers.
- Dense layers run on-chip in bf16 (PE transposes + matmuls); h is
  broadcast between layers with an AllGather collective; log_softmax is
  computed on-chip; the host un-permutes the final (50000, 40) output.
"""

import math
import numpy as np
import ml_dtypes

import concourse.bacc as bacc
import concourse.bass as bass
import concourse.mybir as mybir
from concourse.bass_utils import run_bass_kernel_spmd

N = 50000
E = 800000
NFEAT = 256
NHID = 128
NCLASS = 40
CORES = 8
P = 128
BPC = 49                 # blocks per core
NBLK = CORES * BPC       # 392
RPC = BPC * P            # 6272 rows per core
NPACK = NBLK * P         # 50176 packed rows
NBUF = 32                # gather buffer ring
NSV = 32                 # svals ring

F32 = mybir.dt.float32
BF16 = mybir.dt.bfloat16
I32 = mybir.dt.int32
BF = ml_dtypes.bfloat16


# ----------------------------------------------------------------------------
# Host-side graph packing
# ----------------------------------------------------------------------------

def _pack_graph(rows, cols, vals):
    """Assign nodes to 392 blocks of 128 balancing in-degree sums; build
    per-core per-tile gather indices / dest-locals / vals."""
    indeg = np.bincount(rows, minlength=N).astype(np.int64)
    order = np.argsort(-indeg, kind="stable")
    import heapq
    heap = [(0, b) for b in range(NBLK)]
    heapq.heapify(heap)
    bcount = np.zeros(NBLK, np.int64)
    bsum = np.zeros(NBLK, np.int64)
    pos = np.empty(N, np.int64)
    for v in order:
        while True:
            s, b = heapq.heappop(heap)
            if bcount[b] < P:
                break
        pos[v] = b * P + bcount[b]
        bcount[b] += 1
        bsum[b] += indeg[v]
        if bcount[b] < P:
            heapq.heappush(heap, (bsum[b], b))
    tpb = int(math.ceil(bsum.max() / P))
    d_pos = pos[rows]
    s_pos = pos[cols]
    blk = d_pos // P
    dloc = d_pos % P
    eorder = np.argsort(blk, kind="stable")
    blk_s = blk[eorder]
    dloc_s = (dloc[eorder]).astype(np.float32)
    src_s = s_pos[eorder].astype(np.int32)
    val_s = np.asarray(vals)[eorder].astype(np.float32)
    starts = np.searchsorted(blk_s, np.arange(NBLK + 1))
    T = BPC * tpb
    gidx = np.zeros((CORES, P, T), np.int32)
    gdl = np.zeros((CORES, P, T), np.float32)
    gvl = np.zeros((CORES, P, T), np.float32)
    cap = tpb * P
    for b in range(NBLK):
        lo, hi = starts[b], starts[b + 1]
        n = hi - lo
        assert n <= cap, f"block {b} has {n} edges > cap {cap}"
        c, bc = b // BPC, b % BPC
        sl = np.arange(n)
        tt = bc * tpb + sl // P
        pp = sl % P
        gidx[c, pp, tt] = src_s[lo:hi]
        gdl[c, pp, tt] = dloc_s[lo:hi]
        gvl[c, pp, tt] = val_s[lo:hi]
    return pos, tpb, gidx, gdl, gvl


# ----------------------------------------------------------------------------
# Bass graph
# ----------------------------------------------------------------------------

def _build(tpb):
    T = BPC * tpb
    FDIMS = [NFEAT, NHID, NHID, NHID]
    NINIT = 13 + BPC

    nc = bacc.Bacc("TRN2")

    xt = nc.declare_dram_parameter("xt", [NPACK, NFEAT], BF16, isOutput=False)
    xloc = nc.declare_dram_parameter("xloc", [RPC, NFEAT], BF16, isOutput=False)
    gidx = nc.declare_dram_parameter("gidx", [P, T], I32, isOutput=False)
    gdl = nc.declare_dram_parameter("gdl", [P, T], F32, isOutput=False)
    gvl = nc.declare_dram_parameter("gvl", [P, T], F32, isOutput=False)
    w1p = nc.declare_dram_parameter("w1p", [P, 2 * NHID], BF16, isOutput=False)
    w2p = nc.declare_dram_parameter("w2p", [P, NCLASS], BF16, isOutput=False)
    b1p = nc.declare_dram_parameter("b1p", [P, 1], F32, isOutput=False)
    b2p = nc.declare_dram_parameter("b2p", [NCLASS, 1], F32, isOutput=False)
    d1p = nc.declare_dram_parameter("d1p", [P, NFEAT], F32, isOutput=False)
    hdp = nc.declare_dram_parameter("hdp", [P, 2 * NHID], F32, isOutput=False)
    d2p = nc.declare_dram_parameter("d2p", [P, NHID], F32, isOutput=False)
    iop = nc.declare_dram_parameter("iop", [P, P], F32, isOutput=False)
    idbf = nc.declare_dram_parameter("idbf", [P, P], BF16, isOutput=False)
    idf32 = nc.declare_dram_parameter("idf32", [P, P], F32, isOutput=False)
    outp = nc.declare_dram_parameter("out", [RPC, NCLASS], F32, isOutput=True)

    h_shard = nc.dram_tensor("h_shard", [RPC, NHID], BF16)
    h_table = nc.dram_tensor("h_table", [NPACK, NHID], BF16)

    import contextlib
    ctx = contextlib.ExitStack()
    block = ctx.enter_context(nc.Block())
    sem = {}
    for nm in ["init", "dveinit", "cc", "hw", "os"]:
        sem[nm] = ctx.enter_context(nc.semaphore(nm))
    for L in range(4):
        for nm in ["g", "s", "mm", "t", "u1", "v1", "u2", "v2", "u3", "v3",
                   "u4", "m1", "a2", "m2"]:
            sem[f"{nm}{L}"] = ctx.enter_context(nc.semaphore(f"{nm}{L}"))

    sb = {}
    def S(name, shape, dt):
        sb[name] = ctx.enter_context(nc.sbuf_tensor(name, shape, dt))
        return sb[name]

    gbuf = S("gbuf", [P, NBUF, NFEAT], BF16)
    sval = S("sval", [P, NSV, P], BF16)
    idxs = S("idxs", [P, T], I32)
    dls = S("dls", [P, T], F32)
    vls = S("vls", [P, T], F32)
    w1s = S("w1s", [P, 2 * NHID], BF16)
    w2s = S("w2s", [P, NCLASS], BF16)
    b1s = S("b1s", [P, 1], F32)
    b2s = S("b2s", [NCLASS, 1], F32)
    d1s = S("d1s", [P, NFEAT], F32)
    hds = S("hds", [P, 2 * NHID], F32)
    d2s = S("d2s", [P, NHID], F32)
    ios = S("ios", [P, P], F32)
    idb = S("idb", [P, P], BF16)
    idf = S("idf", [P, P], F32)
    zer = S("zer", [P, NFEAT], BF16)
    xow = S("xow", [P, BPC, NFEAT], BF16)
    how = S("how", [P, BPC, NHID], BF16)
    tmp = S("tmp", [P, NFEAT], BF16)
    hpre = S("hpre", [P, NFEAT], BF16)
    hpT = S("hpT", [P, 2 * NHID], BF16)
    hTs = S("hTs", [P, NHID], BF16)
    p4T = S("p4T", [P, P], BF16)
    oTs = S("oTs", [NCLASS, P], F32)
    nmx = S("nmx", [P, 1], F32)
    sxp = S("sxp", [P, 1], F32)
    lse = S("lse", [P, 1], F32)
    esb = S("esb", [P, NCLASS], F32)
    osb = S("osb", [P, 2, NCLASS], F32)

    ps = {}
    def PS(name, shape, dt=F32):
        ps[name] = ctx.enter_context(nc.psum_tensor(name, shape, dt))
        return ps[name]

    pe1a = PS("pe1a", [P, NFEAT])
    pe1b = PS("pe1b", [P, NFEAT])
    pst = PS("pst", [P, NFEAT], BF16)
    psh = PS("psh", [P, NHID])
    pso = PS("pso", [NCLASS, P])
    po2 = PS("po2", [P, NCLASS])
    pe1 = [pe1a, pe1b]

    tables = [xt, h_table, h_table, h_table]

    # ---------------- Pool: gathers -------------------------------------
    @block.gpsimd
    def _(gp: bass.BassGpSimd):
        gp.wait_ge(sem["init"], 16 * NINIT)
        for L in range(4):
            F = FDIMS[L]
            if L >= 1:
                gp.wait_ge(sem["hw"], 16 * BPC * L)
                gp.collective_compute(
                    "AllGather", mybir.AluOpType.bypass,
                    replica_groups=[list(range(CORES))],
                    ins=[h_shard.ap().opt()],
                    outs=[h_table.ap().opt()],
                ).then_inc(sem["cc"], 1)
                gp.wait_ge(sem["cc"], L)
            for t in range(T):
                if t >= NBUF and t % 8 == 0:
                    gp.wait_ge(sem[f"mm{L}"], t - NBUF + 8)
                gp.indirect_dma_start(
                    out=gbuf[:, t % NBUF, :F],
                    out_offset=None,
                    in_=tables[L][:, :],
                    in_offset=bass.IndirectOffsetOnAxis(ap=idxs[:, t:t + 1], axis=0),
                ).then_inc(sem[f"g{L}"], 16)

    # ---------------- PE ------------------------------------------------
    @block.tensor
    def _(pe: bass.BassTensorEngine):
        pe.wait_ge(sem["init"], 16 * NINIT)
        pe.wait_ge(sem["dveinit"], 1)

        def tail(L, b):
            if b < 0:
                return
            if L == 0:
                pe.wait_ge(sem["u10"], b + 1)   # hpre(b) ready (DVE)
                pe.wait_ge(sem["u40"], b)       # ACT done reading pst of b-1
                pe.transpose(out=pst[:, 0:P], in_=hpre[:, 0:P], identity=idb[:, :])
                pe.transpose(out=pst[:, P:2 * P], in_=hpre[:, P:2 * P], identity=idb[:, :]).then_inc(sem["v10"], 1)
                pe.wait_ge(sem["u20"], b + 1)   # hpT copied (ACT)
                pe.matmul(out=psh[:, :], lhsT=w1s[:, 0:NHID], rhs=hpT[:, 0:NHID],
                          start=True, stop=False, skip_group_check=True)
                pe.matmul(out=psh[:, :], lhsT=w1s[:, NHID:2 * NHID], rhs=hpT[:, NHID:2 * NHID],
                          start=False, stop=True, skip_group_check=True).then_inc(sem["v20"], 1)
                pe.wait_ge(sem["u30"], b + 1)   # hT relu'd (ACT)
                pe.transpose(out=pst[:, 0:P], in_=hTs[:, :], identity=idb[:, :]).then_inc(sem["v30"], 1)
            elif L == 3:
                pe.wait_ge(sem["u13"], b + 1)   # pre4(b) ready (DVE)
                pe.wait_ge(sem["u23"], b)       # ACT done reading pst of b-1
                pe.transpose(out=pst[:, 0:P], in_=hpre[:, 0:P], identity=idb[:, :]).then_inc(sem["v13"], 1)
                pe.wait_ge(sem["u23"], b + 1)   # p4T copied (ACT)
                pe.matmul(out=pso[:, :], lhsT=w2s[:, :], rhs=p4T[:, :],
                          start=True, stop=True, skip_group_check=True).then_inc(sem["v23"], 1)
                pe.wait_ge(sem["u33"], b + 1)   # oTs relu'd (ACT)
                pe.wait_ge(sem["a23"], b)       # ACT exp of b-1 done reading po2
                pe.wait_ge(sem["m23"], b)       # DVE final of b-1 done reading po2
                pe.transpose(out=po2[:, :], in_=oTs[:, :], identity=idf[0:NCLASS, 0:NCLASS]).then_inc(sem["v33"], 1)

        for L in range(4):
            F = FDIMS[L]
            for b in range(BPC):
                if b >= 2:
                    pe.wait_ge(sem[f"t{L}"], b - 1)
                elif L >= 1:
                    pe.wait_ge(sem[f"t{L-1}"], BPC)
                pe.matmul(out=pe1[b % 2][:, :F], lhsT=zer[:, 0:P], rhs=zer[:, :F],
                          start=True, stop=False, skip_group_check=True)
                for k in range(tpb):
                    t = b * tpb + k
                    pe.wait_ge(sem[f"g{L}"], 16 * (t + 1))
                    pe.wait_ge(sem[f"s{L}"], t + 1)
                    pe.matmul(out=pe1[b % 2][:, :F], lhsT=sval[:, t % NSV, :],
                              rhs=gbuf[:, t % NBUF, :F], start=False,
                              stop=(k == tpb - 1),
                              skip_group_check=True).then_inc(sem[f"mm{L}"], 1)
                tail(L, b - 1)
            tail(L, BPC - 1)

    # ---------------- DVE -----------------------------------------------
    @block.vector
    def _(dv: bass.BassVectorEngine):
        dv.memset(zer[:, :], 0)
        dv.sem_inc(sem["dveinit"], 1)
        dv.wait_ge(sem["init"], 16 * NINIT)

        def tail(L, b):
            if b < 0:
                return
            dv.wait_ge(sem[f"mm{L}"], (b + 1) * tpb)
            F = FDIMS[L]
            pp = pe1[b % 2]
            if L == 0:
                dv.tensor_tensor(out=tmp[:, :F], in0=pp[:, :F], in1=d1s[:, :F],
                                 op=mybir.AluOpType.mult).then_inc(sem["t0"], 1)
                dv.wait_ge(sem["v10"], b)       # PE done transposing hpre(b-1)
                dv.tensor_tensor(out=hpre[:, :F], in0=xow[:, b, :], in1=tmp[:, :F],
                                 op=mybir.AluOpType.subtract).then_inc(sem["u10"], 1)
            elif L in (1, 2):
                dv.tensor_tensor(out=tmp[:, :F], in0=pp[:, :F], in1=hds[:, (L - 1) * NHID:L * NHID],
                                 op=mybir.AluOpType.mult).then_inc(sem[f"t{L}"], 1)
                dv.wait_ge(sem[f"u2{L}"], b)    # ACT done reading hpre(b-1)
                dv.tensor_tensor(out=hpre[:, 0:F], in0=how[:, b, :], in1=tmp[:, :F],
                                 op=mybir.AluOpType.subtract).then_inc(sem[f"u1{L}"], 1)
            else:
                dv.tensor_tensor(out=tmp[:, :F], in0=pp[:, :F], in1=d2s[:, :F],
                                 op=mybir.AluOpType.mult).then_inc(sem["t3"], 1)
                dv.wait_ge(sem["v13"], b)       # PE done transposing hpre(b-1)
                dv.tensor_tensor(out=hpre[:, 0:F], in0=how[:, b, :], in1=tmp[:, :F],
                                 op=mybir.AluOpType.subtract).then_inc(sem["u13"], 1)
                dv.wait_ge(sem["v33"], b + 1)
                dv.tensor_reduce(out=nmx[:, :], in_=po2[:, :], axis=mybir.AxisListType.X,
                                 op=mybir.AluOpType.max, negate=True).then_inc(sem["m13"], 1)
                dv.wait_ge(sem["a23"], b + 1)
                if b >= 2:
                    dv.wait_ge(sem["os"], 16 * (b - 1))  # osb slot free
                dv.tensor_scalar(out=osb[:, b % 2, :], in0=po2[:, :], scalar1=nmx[:, :1],
                                 scalar2=lse[:, :1], op0=mybir.AluOpType.add,
                                 op1=mybir.AluOpType.subtract).then_inc(sem["m23"], 1)

        for L in range(4):
            if L >= 1:
                dv.wait_ge(sem[f"mm{L-1}"], T)
            for b in range(BPC):
                for k in range(tpb):
                    t = b * tpb + k
                    if t >= NSV and t % 8 == 0:
                        dv.wait_ge(sem[f"mm{L}"], t - NSV + 8)
                    dv.tensor_scalar(out=sval[:, t % NSV, :], in0=ios[:, :],
                                     scalar1=dls[:, t:t + 1], scalar2=vls[:, t:t + 1],
                                     op0=mybir.AluOpType.is_equal,
                                     op1=mybir.AluOpType.mult).then_inc(sem[f"s{L}"], 1)
                tail(L, b - 1)
            tail(L, BPC - 1)

    # ---------------- ACT -----------------------------------------------
    @block.scalar
    def _(ac: bass.BassScalarEngine):
        AF = mybir.ActivationFunctionType
        for src, dst in [(gidx, idxs), (gdl, dls), (gvl, vls), (b1p, b1s),
                         (b2p, b2s), (d1p, d1s), (d2p, d2s), (iop, ios),
                         (idbf, idb), (idf32, idf), (w2p, w2s)]:
            ac.dma_start(out=dst[:, :], in_=src[:, :]).then_inc(sem["init"], 16)
        ac.dma_start(out=w1s[:, :], in_=w1p[:, :]).then_inc(sem["init"], 16)
        ac.dma_start(out=hds[:, :], in_=hdp[:, :]).then_inc(sem["init"], 16)
        for b in range(BPC):
            ac.dma_start(out=xow[:, b, :], in_=xloc[b * P:(b + 1) * P, :]).then_inc(sem["init"], 16)

        def tail(L, b):
            if b < 0:
                return
            if L == 0:
                ac.wait_ge(sem["v10"], b + 1)
                ac.activation(out=hpT[:, :], in_=pst[:, :],
                              func=AF.Copy).then_inc(sem["u20"], 1)
                ac.wait_ge(sem["v20"], b + 1)
                ac.activation(out=hTs[:, :], in_=psh[:, :], func=AF.Relu,
                              bias=b1s[:, :1]).then_inc(sem["u30"], 1)
                ac.wait_ge(sem["v30"], b + 1)
                ac.activation(out=how[:, b, :], in_=pst[:, 0:P], func=AF.Copy).then_inc(sem["u40"], 1)
                ac.dma_start(out=h_shard[b * P:(b + 1) * P, :], in_=how[:, b, :]).then_inc(sem["hw"], 16)
            elif L in (1, 2):
                ac.wait_ge(sem[f"u1{L}"], b + 1)
                ac.activation(out=how[:, b, :], in_=hpre[:, 0:NHID],
                              func=AF.Relu).then_inc(sem[f"u2{L}"], 1)
                ac.dma_start(out=h_shard[b * P:(b + 1) * P, :], in_=how[:, b, :]).then_inc(sem["hw"], 16)
            else:
                ac.wait_ge(sem["v13"], b + 1)
                ac.activation(out=p4T[:, :], in_=pst[:, 0:P], func=AF.Copy).then_inc(sem["u23"], 1)
                ac.wait_ge(sem["v23"], b + 1)
                ac.activation(out=oTs[:, :], in_=pso[:, :], func=AF.Relu,
                              bias=b2s[:, :1]).then_inc(sem["u33"], 1)
                ac.wait_ge(sem["m13"], b + 1)
                ac.activation(out=esb[:, :], in_=po2[:, :], func=AF.Exp,
                              bias=nmx[:, :1], accum_out=sxp[:, :1])
                ac.activation(out=lse[:, :], in_=sxp[:, :], func=AF.Ln).then_inc(sem["a23"], 1)
                ac.wait_ge(sem["m23"], b + 1)
                ac.dma_start(out=outp[b * P:(b + 1) * P, :], in_=osb[:, b % 2, :]).then_inc(sem["os"], 16)

        for L in range(4):
            if L in (1, 2):
                ac.wait_ge(sem["cc"], L)
            for b in range(BPC):
                tail(L, b - 1)
            tail(L, BPC - 1)
        ac.wait_ge(sem["os"], 16 * BPC)

    ctx.close()
    nc.compile()
    return nc


_CACHE = {}


def kernel(x, rows, cols, vals, diag1, W1, b1, hidden_diags, diag2, W2, b2):
    x = np.asarray(x)
    rows = np.asarray(rows).astype(np.int64)
    cols = np.asarray(cols).astype(np.int64)
    vals = np.asarray(vals)
    pos, tpb, gidx, gdl, gvl = _pack_graph(rows, cols, vals)

    if tpb not in _CACHE:
        _CACHE[tpb] = _build(tpb)
    nc = _CACHE[tpb]

    x_packed = np.zeros((NPACK, NFEAT), BF)
    x_packed[pos] = np.asarray(x).astype(BF)
    iota = np.tile(np.arange(P, dtype=np.float32)[None, :], (P, 1))
    ident = np.eye(P, dtype=np.float32)
    d1 = np.tile((np.asarray(diag1) + 1.0).astype(np.float32)[None, :], (P, 1))
    hd = np.tile(np.asarray(hidden_diags).astype(np.float32).reshape(1, -1), (P, 1))
    d2 = np.tile((np.asarray(diag2) + 1.0).astype(np.float32)[None, :], (P, 1))
    w1 = np.asarray(W1).astype(BF).reshape(2, P, NHID).transpose(1, 0, 2).reshape(P, 2 * NHID)
    w2 = np.asarray(W2).astype(BF)
    b1c = np.asarray(b1).astype(np.float32)[:, None]
    b2c = np.asarray(b2).astype(np.float32)[:, None]

    in_maps = []
    for c in range(CORES):
        in_maps.append({
            "xt": x_packed,
            "xloc": x_packed[c * RPC:(c + 1) * RPC],
            "gidx": gidx[c], "gdl": gdl[c], "gvl": gvl[c],
            "w1p": w1, "w2p": w2, "b1p": b1c, "b2p": b2c,
            "d1p": d1, "hdp": hd, "d2p": d2,
            "iop": iota, "idbf": ident.astype(BF), "idf32": ident,
        })

    res = run_bass_kernel_spmd(nc, in_maps, core_ids=list(range(CORES)))
    out_packed = np.concatenate([res.results[c]["out"] for c in range(CORES)], axis=0)
    return out_packed[pos].astype(np.float32)


# revision 8
# speedup vs baseline: 1.0454x; 1.0454x over previous
"""AdaGNN on 8 TRN2 NeuronCores (Bass, SPMD).

Strategy (node sharding, replicated graph tables):
- Host packs the 50000 nodes into 8 cores x 49 blocks x 128 rows via a
  load-balancing permutation (block in-degree sums <= TPB*128).
- spmm per 128-slot tile: one K=1 indirect DMA gathers the 128 source rows
  (one per partition) from the DRAM feature table; the DVE builds a
  (128 slots x 128 rows) one-hot-times-val matrix; the PE accumulates
  psum_block += S_vals.T @ gathered.  Graph structure (gather indices,
  dest-locals, vals) is identical for all 4 spmm layers.
- Dense layers run on-chip in bf16 (PE transposes + matmuls); h is
  broadcast between layers with an AllGather collective; log_softmax is
  computed on-chip; the host un-permutes the final (50000, 40) output.
"""

import math
import numpy as np
import ml_dtypes

import concourse.bacc as bacc
import concourse.bass as bass
import concourse.mybir as mybir
from concourse.bass_utils import run_bass_kernel_spmd

N = 50000
E = 800000
NFEAT = 256
NHID = 128
NCLASS = 40
CORES = 8
P = 128
BPC = 49                 # blocks per core
NBLK = CORES * BPC       # 392
RPC = BPC * P            # 6272 rows per core
NPACK = NBLK * P         # 50176 packed rows
NBUF = 32                # gather buffer ring
NSV = 32                 # svals ring

F32 = mybir.dt.float32
BF16 = mybir.dt.bfloat16
I32 = mybir.dt.int32
BF = ml_dtypes.bfloat16


# ----------------------------------------------------------------------------
# Host-side graph packing
# ----------------------------------------------------------------------------

def _pack_graph(rows, cols, vals):
    """Assign nodes to 392 blocks of 128 balancing in-degree sums; build
    per-core per-tile gather indices / dest-locals / vals."""
    indeg = np.bincount(rows, minlength=N).astype(np.int64)
    order = np.argsort(-indeg, kind="stable")
    import heapq
    heap = [(0, b) for b in range(NBLK)]
    heapq.heapify(heap)
    bcount = np.zeros(NBLK, np.int64)
    bsum = np.zeros(NBLK, np.int64)
    pos = np.empty(N, np.int64)
    for v in order:
        while True:
            s, b = heapq.heappop(heap)
            if bcount[b] < P:
                break
        pos[v] = b * P + bcount[b]
        bcount[b] += 1
        bsum[b] += indeg[v]
        if bcount[b] < P:
            heapq.heappush(heap, (bsum[b], b))
    tpb = int(math.ceil(bsum.max() / P))
    # table position: region A = all cores' blocks 0..SPLIT-1, region B = rest.
    # pos is (block*P + loc) with block = c*BPC + b; table_pos reorders rows so
    # each AllGather half writes a contiguous table region.
    SPLIT = 40
    blk_all = pos // P
    loc_all = pos % P
    c_all = blk_all // BPC
    b_all = blk_all % BPC
    tpos = np.where(
        b_all < SPLIT,
        (SPLIT * P) * c_all + P * b_all + loc_all,
        CORES * SPLIT * P + (BPC - SPLIT) * P * c_all + P * (b_all - SPLIT) + loc_all,
    )
    d_pos = pos[rows]
    s_pos = tpos[cols]
    blk = d_pos // P
    dloc = d_pos % P
    eorder = np.argsort(blk, kind="stable")
    blk_s = blk[eorder]
    dloc_s = (dloc[eorder]).astype(np.float32)
    src_s = s_pos[eorder].astype(np.int32)
    val_s = np.asarray(vals)[eorder].astype(np.float32)
    starts = np.searchsorted(blk_s, np.arange(NBLK + 1))
    T = BPC * tpb
    gidx = np.zeros((CORES, P, T), np.int32)
    gdl = np.zeros((CORES, P, T), np.float32)
    gvl = np.zeros((CORES, P, T), np.float32)
    cap = tpb * P
    for b in range(NBLK):
        lo, hi = starts[b], starts[b + 1]
        n = hi - lo
        assert n <= cap, f"block {b} has {n} edges > cap {cap}"
        c, bc = b // BPC, b % BPC
        sl = np.arange(n)
        tt = bc * tpb + sl // P
        pp = sl % P
        gidx[c, pp, tt] = src_s[lo:hi]
        gdl[c, pp, tt] = dloc_s[lo:hi]
        gvl[c, pp, tt] = val_s[lo:hi]
    return pos, tpos, tpb, gidx, gdl, gvl


# ----------------------------------------------------------------------------
# Bass graph
# ----------------------------------------------------------------------------

def _build(tpb):
    T = BPC * tpb
    SPLIT = 40
    FDIMS = [NFEAT, NHID, NHID, NHID]
    NINIT = 13 + BPC

    nc = bacc.Bacc("TRN2")

    xt = nc.declare_dram_parameter("xt", [NPACK, NFEAT], BF16, isOutput=False)
    xloc = nc.declare_dram_parameter("xloc", [RPC, NFEAT], BF16, isOutput=False)
    gidx = nc.declare_dram_parameter("gidx", [P, T], I32, isOutput=False)
    gdl = nc.declare_dram_parameter("gdl", [P, T], F32, isOutput=False)
    gvl = nc.declare_dram_parameter("gvl", [P, T], F32, isOutput=False)
    w1p = nc.declare_dram_parameter("w1p", [P, 2 * NHID], BF16, isOutput=False)
    w2p = nc.declare_dram_parameter("w2p", [P, NCLASS], BF16, isOutput=False)
    b1p = nc.declare_dram_parameter("b1p", [P, 1], F32, isOutput=False)
    b2p = nc.declare_dram_parameter("b2p", [NCLASS, 1], F32, isOutput=False)
    d1p = nc.declare_dram_parameter("d1p", [P, NFEAT], F32, isOutput=False)
    hdp = nc.declare_dram_parameter("hdp", [P, 2 * NHID], F32, isOutput=False)
    d2p = nc.declare_dram_parameter("d2p", [P, NHID], F32, isOutput=False)
    iop = nc.declare_dram_parameter("iop", [P, P], F32, isOutput=False)
    idbf = nc.declare_dram_parameter("idbf", [P, P], BF16, isOutput=False)
    idf32 = nc.declare_dram_parameter("idf32", [P, P], F32, isOutput=False)
    outp = nc.declare_dram_parameter("out", [RPC, NCLASS], F32, isOutput=True)

    h_shard = nc.dram_tensor("h_shard", [RPC, NHID], BF16)
    h_tA = nc.dram_tensor("h_tA", [NPACK, NHID], BF16)
    h_tB = nc.dram_tensor("h_tB", [NPACK, NHID], BF16)

    import contextlib
    ctx = contextlib.ExitStack()
    block = ctx.enter_context(nc.Block())
    sem = {}
    for nm in ["init", "dveinit", "cc", "hw", "os"]:
        sem[nm] = ctx.enter_context(nc.semaphore(nm))
    for L in range(4):
        for nm in ["g", "s", "mm", "t", "u1", "v1", "u2", "v2", "u3", "v3",
                   "u4", "m1", "a2", "m2"]:
            sem[f"{nm}{L}"] = ctx.enter_context(nc.semaphore(f"{nm}{L}"))

    sb = {}
    def S(name, shape, dt):
        sb[name] = ctx.enter_context(nc.sbuf_tensor(name, shape, dt))
        return sb[name]

    gbuf = S("gbuf", [P, NBUF, NFEAT], BF16)
    sval = S("sval", [P, NSV, P], BF16)
    idxs = S("idxs", [P, T], I32)
    dls = S("dls", [P, T], F32)
    vls = S("vls", [P, T], F32)
    w1s = S("w1s", [P, 2 * NHID], BF16)
    w2s = S("w2s", [P, NCLASS], BF16)
    b1s = S("b1s", [P, 1], F32)
    b2s = S("b2s", [NCLASS, 1], F32)
    d1s = S("d1s", [P, NFEAT], F32)
    hds = S("hds", [P, 2 * NHID], F32)
    d2s = S("d2s", [P, NHID], F32)
    ios = S("ios", [P, P], F32)
    idb = S("idb", [P, P], BF16)
    idf = S("idf", [P, P], F32)
    zer = S("zer", [P, NFEAT], BF16)
    xow = S("xow", [P, BPC, NFEAT], BF16)
    how = S("how", [P, BPC, NHID], BF16)
    tmp = S("tmp", [P, NFEAT], BF16)
    hpre = S("hpre", [P, NFEAT], BF16)
    hpT = S("hpT", [P, 2 * NHID], BF16)
    hTs = S("hTs", [P, NHID], BF16)
    p4T = S("p4T", [P, P], BF16)
    oTs = S("oTs", [NCLASS, P], F32)
    nmx = S("nmx", [P, 1], F32)
    sxp = S("sxp", [P, 1], F32)
    lse = S("lse", [P, 1], F32)
    esb = S("esb", [P, NCLASS], F32)
    osb = S("osb", [P, 2, NCLASS], F32)

    ps = {}
    def PS(name, shape, dt=F32):
        ps[name] = ctx.enter_context(nc.psum_tensor(name, shape, dt))
        return ps[name]

    pe1a = PS("pe1a", [P, NFEAT])
    pe1b = PS("pe1b", [P, NFEAT])
    pst = PS("pst", [P, NFEAT], BF16)
    psh = PS("psh", [P, NHID])
    pso = PS("pso", [NCLASS, P])
    po2 = PS("po2", [P, NCLASS])
    pe1 = [pe1a, pe1b]

    tables = [xt, h_tA, h_tB, h_tA]

    # ---------------- Pool: gathers -------------------------------------
    @block.gpsimd
    def _(gp: bass.BassGpSimd):
        gp.wait_ge(sem["init"], 16 * NINIT)
        for L in range(4):
            F = FDIMS[L]
            if L >= 1:
                gp.wait_ge(sem["hw"], 16 * BPC * L)
                gp.collective_compute(
                    "AllGather", mybir.AluOpType.bypass,
                    replica_groups=[list(range(CORES))],
                    ins=[h_shard[SPLIT * P:BPC * P, :].opt()],
                    outs=[tables[L][CORES * SPLIT * P:NPACK, :].opt()],
                ).then_inc(sem["cc"], 1)
                gp.wait_ge(sem["cc"], 2 * L)
            for t in range(T):
                if L < 3 and t == 704:
                    gp.wait_ge(sem["hw"], 16 * (BPC * L + SPLIT))
                    gp.collective_compute(
                        "AllGather", mybir.AluOpType.bypass,
                        replica_groups=[list(range(CORES))],
                        ins=[h_shard[0:SPLIT * P, :].opt()],
                        outs=[tables[L + 1][0:CORES * SPLIT * P, :].opt()],
                    ).then_inc(sem["cc"], 1)
                if t >= NBUF and t % 8 == 0:
                    gp.wait_ge(sem[f"mm{L}"], t - NBUF + 8)
                gp.indirect_dma_start(
                    out=gbuf[:, t % NBUF, :F],
                    out_offset=None,
                    in_=tables[L][:, :],
                    in_offset=bass.IndirectOffsetOnAxis(ap=idxs[:, t:t + 1], axis=0),
                ).then_inc(sem[f"g{L}"], 16)

    # ---------------- PE ------------------------------------------------
    @block.tensor
    def _(pe: bass.BassTensorEngine):
        pe.wait_ge(sem["init"], 16 * NINIT)
        pe.wait_ge(sem["dveinit"], 1)

        def tail(L, b):
            if b < 0:
                return
            if L == 0:
                pe.wait_ge(sem["u10"], b + 1)   # hpre(b) ready (DVE)
                pe.wait_ge(sem["u40"], b)       # ACT done reading pst of b-1
                pe.transpose(out=pst[:, 0:P], in_=hpre[:, 0:P], identity=idb[:, :])
                pe.transpose(out=pst[:, P:2 * P], in_=hpre[:, P:2 * P], identity=idb[:, :]).then_inc(sem["v10"], 1)
                pe.wait_ge(sem["u20"], b + 1)   # hpT copied (ACT)
                pe.matmul(out=psh[:, :], lhsT=w1s[:, 0:NHID], rhs=hpT[:, 0:NHID],
                          start=True, stop=False, skip_group_check=True)
                pe.matmul(out=psh[:, :], lhsT=w1s[:, NHID:2 * NHID], rhs=hpT[:, NHID:2 * NHID],
                          start=False, stop=True, skip_group_check=True).then_inc(sem["v20"], 1)
                pe.wait_ge(sem["u30"], b + 1)   # hT relu'd (ACT)
                pe.transpose(out=pst[:, 0:P], in_=hTs[:, :], identity=idb[:, :]).then_inc(sem["v30"], 1)
            elif L == 3:
                pe.wait_ge(sem["u13"], b + 1)   # pre4(b) ready (DVE)
                pe.wait_ge(sem["u23"], b)       # ACT done reading pst of b-1
                pe.transpose(out=pst[:, 0:P], in_=hpre[:, 0:P], identity=idb[:, :]).then_inc(sem["v13"], 1)
                pe.wait_ge(sem["u23"], b + 1)   # p4T copied (ACT)
                pe.matmul(out=pso[:, :], lhsT=w2s[:, :], rhs=p4T[:, :],
                          start=True, stop=True, skip_group_check=True).then_inc(sem["v23"], 1)
                pe.wait_ge(sem["u33"], b + 1)   # oTs relu'd (ACT)
                pe.wait_ge(sem["a23"], b)       # ACT exp of b-1 done reading po2
                pe.wait_ge(sem["m23"], b)       # DVE final of b-1 done reading po2
                pe.transpose(out=po2[:, :], in_=oTs[:, :], identity=idf[0:NCLASS, 0:NCLASS]).then_inc(sem["v33"], 1)

        for L in range(4):
            F = FDIMS[L]
            for b in range(BPC):
                if b >= 2:
                    pe.wait_ge(sem[f"t{L}"], b - 1)
                elif L >= 1:
                    pe.wait_ge(sem[f"t{L-1}"], BPC)
                pe.matmul(out=pe1[b % 2][:, :F], lhsT=zer[:, 0:P], rhs=zer[:, :F],
                          start=True, stop=False, skip_group_check=True)
                for k in range(tpb):
                    t = b * tpb + k
                    pe.wait_ge(sem[f"g{L}"], 16 * (t + 1))
                    pe.wait_ge(sem[f"s{L}"], t + 1)
                    pe.matmul(out=pe1[b % 2][:, :F], lhsT=sval[:, t % NSV, :],
                              rhs=gbuf[:, t % NBUF, :F], start=False,
                              stop=(k == tpb - 1),
                              skip_group_check=True).then_inc(sem[f"mm{L}"], 1)
                tail(L, b - 1)
            tail(L, BPC - 1)

    # ---------------- DVE -----------------------------------------------
    @block.vector
    def _(dv: bass.BassVectorEngine):
        dv.memset(zer[:, :], 0)
        dv.sem_inc(sem["dveinit"], 1)
        dv.wait_ge(sem["init"], 16 * NINIT)

        def tail(L, b):
            if b < 0:
                return
            dv.wait_ge(sem[f"mm{L}"], (b + 1) * tpb)
            F = FDIMS[L]
            pp = pe1[b % 2]
            if L == 0:
                dv.tensor_tensor(out=tmp[:, :F], in0=pp[:, :F], in1=d1s[:, :F],
                                 op=mybir.AluOpType.mult).then_inc(sem["t0"], 1)
                dv.wait_ge(sem["v10"], b)       # PE done transposing hpre(b-1)
                dv.tensor_tensor(out=hpre[:, :F], in0=xow[:, b, :], in1=tmp[:, :F],
                                 op=mybir.AluOpType.subtract).then_inc(sem["u10"], 1)
            elif L in (1, 2):
                dv.tensor_tensor(out=tmp[:, :F], in0=pp[:, :F], in1=hds[:, (L - 1) * NHID:L * NHID],
                                 op=mybir.AluOpType.mult).then_inc(sem[f"t{L}"], 1)
                dv.wait_ge(sem[f"u2{L}"], b)    # ACT done reading hpre(b-1)
                dv.tensor_tensor(out=hpre[:, 0:F], in0=how[:, b, :], in1=tmp[:, :F],
                                 op=mybir.AluOpType.subtract).then_inc(sem[f"u1{L}"], 1)
            else:
                dv.tensor_tensor(out=tmp[:, :F], in0=pp[:, :F], in1=d2s[:, :F],
                                 op=mybir.AluOpType.mult).then_inc(sem["t3"], 1)
                dv.wait_ge(sem["v13"], b)       # PE done transposing hpre(b-1)
                dv.tensor_tensor(out=hpre[:, 0:F], in0=how[:, b, :], in1=tmp[:, :F],
                                 op=mybir.AluOpType.subtract).then_inc(sem["u13"], 1)
                dv.wait_ge(sem["v33"], b + 1)
                dv.tensor_reduce(out=nmx[:, :], in_=po2[:, :], axis=mybir.AxisListType.X,
                                 op=mybir.AluOpType.max, negate=True).then_inc(sem["m13"], 1)
                dv.wait_ge(sem["a23"], b + 1)
                if b >= 2:
                    dv.wait_ge(sem["os"], 16 * (b - 1))  # osb slot free
                dv.tensor_scalar(out=osb[:, b % 2, :], in0=po2[:, :], scalar1=nmx[:, :1],
                                 scalar2=lse[:, :1], op0=mybir.AluOpType.add,
                                 op1=mybir.AluOpType.subtract).then_inc(sem["m23"], 1)

        for L in range(4):
            if L >= 1:
                dv.wait_ge(sem[f"mm{L-1}"], T)
            for b in range(BPC):
                for k in range(tpb):
                    t = b * tpb + k
                    if t >= NSV and t % 8 == 0:
                        dv.wait_ge(sem[f"mm{L}"], t - NSV + 8)
                    dv.tensor_scalar(out=sval[:, t % NSV, :], in0=ios[:, :],
                                     scalar1=dls[:, t:t + 1], scalar2=vls[:, t:t + 1],
                                     op0=mybir.AluOpType.is_equal,
                                     op1=mybir.AluOpType.mult).then_inc(sem[f"s{L}"], 1)
                tail(L, b - 1)
            tail(L, BPC - 1)

    # ---------------- ACT -----------------------------------------------
    @block.scalar
    def _(ac: bass.BassScalarEngine):
        AF = mybir.ActivationFunctionType
        for src, dst in [(gidx, idxs), (gdl, dls), (gvl, vls), (b1p, b1s),
                         (b2p, b2s), (d1p, d1s), (d2p, d2s), (iop, ios),
                         (idbf, idb), (idf32, idf), (w2p, w2s)]:
            ac.dma_start(out=dst[:, :], in_=src[:, :]).then_inc(sem["init"], 16)
        ac.dma_start(out=w1s[:, :], in_=w1p[:, :]).then_inc(sem["init"], 16)
        ac.dma_start(out=hds[:, :], in_=hdp[:, :]).then_inc(sem["init"], 16)
        for b in range(BPC):
            ac.dma_start(out=xow[:, b, :], in_=xloc[b * P:(b + 1) * P, :]).then_inc(sem["init"], 16)

        def tail(L, b):
            if b < 0:
                return
            if L == 0:
                ac.wait_ge(sem["v10"], b + 1)
                ac.activation(out=hpT[:, :], in_=pst[:, :],
                              func=AF.Copy).then_inc(sem["u20"], 1)
                ac.wait_ge(sem["v20"], b + 1)
                ac.activation(out=hTs[:, :], in_=psh[:, :], func=AF.Relu,
                              bias=b1s[:, :1]).then_inc(sem["u30"], 1)
                ac.wait_ge(sem["v30"], b + 1)
                ac.activation(out=how[:, b, :], in_=pst[:, 0:P], func=AF.Copy).then_inc(sem["u40"], 1)
                ac.dma_start(out=h_shard[b * P:(b + 1) * P, :], in_=how[:, b, :]).then_inc(sem["hw"], 16)
            elif L in (1, 2):
                ac.wait_ge(sem[f"u1{L}"], b + 1)
                ac.activation(out=how[:, b, :], in_=hpre[:, 0:NHID],
                              func=AF.Relu).then_inc(sem[f"u2{L}"], 1)
                ac.dma_start(out=h_shard[b * P:(b + 1) * P, :], in_=how[:, b, :]).then_inc(sem["hw"], 16)
            else:
                ac.wait_ge(sem["v13"], b + 1)
                ac.activation(out=p4T[:, :], in_=pst[:, 0:P], func=AF.Copy).then_inc(sem["u23"], 1)
                ac.wait_ge(sem["v23"], b + 1)
                ac.activation(out=oTs[:, :], in_=pso[:, :], func=AF.Relu,
                              bias=b2s[:, :1]).then_inc(sem["u33"], 1)
                ac.wait_ge(sem["m13"], b + 1)
                ac.activation(out=esb[:, :], in_=po2[:, :], func=AF.Exp,
                              bias=nmx[:, :1], accum_out=sxp[:, :1])
                ac.activation(out=lse[:, :], in_=sxp[:, :], func=AF.Ln).then_inc(sem["a23"], 1)
                ac.wait_ge(sem["m23"], b + 1)
                ac.dma_start(out=outp[b * P:(b + 1) * P, :], in_=osb[:, b % 2, :]).then_inc(sem["os"], 16)

        for L in range(4):
            if L in (1, 2):
                ac.wait_ge(sem["cc"], 2 * L)
            for b in range(BPC):
                tail(L, b - 1)
            tail(L, BPC - 1)
        ac.wait_ge(sem["os"], 16 * BPC)

    ctx.close()
    nc.compile()
    return nc


_CACHE = {}


def kernel(x, rows, cols, vals, diag1, W1, b1, hidden_diags, diag2, W2, b2):
    x = np.asarray(x)
    rows = np.asarray(rows).astype(np.int64)
    cols = np.asarray(cols).astype(np.int64)
    vals = np.asarray(vals)
    pos, tpos, tpb, gidx, gdl, gvl = _pack_graph(rows, cols, vals)

    if tpb not in _CACHE:
        _CACHE[tpb] = _build(tpb)
    nc = _CACHE[tpb]

    x_packed = np.zeros((NPACK, NFEAT), BF)
    x_packed[tpos] = np.asarray(x).astype(BF)
    x_shard = np.zeros((NPACK, NFEAT), BF)
    x_shard[pos] = np.asarray(x).astype(BF)
    iota = np.tile(np.arange(P, dtype=np.float32)[None, :], (P, 1))
    ident = np.eye(P, dtype=np.float32)
    d1 = np.tile((np.asarray(diag1) + 1.0).astype(np.float32)[None, :], (P, 1))
    hd = np.tile(np.asarray(hidden_diags).astype(np.float32).reshape(1, -1), (P, 1))
    d2 = np.tile((np.asarray(diag2) + 1.0).astype(np.float32)[None, :], (P, 1))
    w1 = np.asarray(W1).astype(BF).reshape(2, P, NHID).transpose(1, 0, 2).reshape(P, 2 * NHID)
    w2 = np.asarray(W2).astype(BF)
    b1c = np.asarray(b1).astype(np.float32)[:, None]
    b2c = np.asarray(b2).astype(np.float32)[:, None]

    in_maps = []
    for c in range(CORES):
        in_maps.append({
            "xt": x_packed,
            "xloc": x_shard[c * RPC:(c + 1) * RPC],
            "gidx": gidx[c], "gdl": gdl[c], "gvl": gvl[c],
            "w1p": w1, "w2p": w2, "b1p": b1c, "b2p": b2c,
            "d1p": d1, "hdp": hd, "d2p": d2,
            "iop": iota, "idbf": ident.astype(BF), "idf32": ident,
        })

    res = run_bass_kernel_spmd(nc, in_maps, core_ids=list(range(CORES)))
    out_packed = np.concatenate([res.results[c]["out"] for c in range(CORES)], axis=0)
    return out_packed[pos].astype(np.float32)
